# revision 13
# baseline (speedup 1.0000x reference)
# Trainium2 Bass kernel for nn_CALayer_31447750541610 (channel-attention layer).
#
# Math (per batch image, C=64 channels, n=H*W pixels):
#   pool[c] = mean_n x[c,n]
#   so[c]   = sum_d corr[c,d] * Wrow[c,d] + brow[c],  corr = x @ x.T / n
#   y       = pool + so
#   g       = sigmoid(relu(y @ W1.T + b1) @ W2.T + b2)
#   out     = x * g[c]
#
# Key rewrite: so[c] = (1/n) sum_n x[c,n] * V[c,n] with V = Wrow @ x, so the
# C x C Gram matrix is never materialized and x is consumed in its natural
# channel-major layout (no transpose). Folding pool in:
#   y = (1/n) sum_n x[c,n] * (V[c,n] + 1) + brow[c]
#
# Distribution: pure data parallel, B=16 batches over 8 cores; each core's 2
# batches are stacked into the 128 SBUF partitions (2 x 64 channels) so every
# engine op runs at full width. The first NCACHE pixel-chunks stay resident in
# SBUF after pass 1, so pass 2 (out = x * g) only re-reads the tail from HBM.

import ml_dtypes
import numpy as np

import concourse.bacc as bacc
import concourse.tile as tile
import concourse.mybir as mybir
from concourse.bass_utils import run_bass_kernel_spmd

B, C, H, W = 16, 64, 256, 256
N = H * W                  # 65536 pixels
RED = 16
NCORES = 8
BPC = B // NCORES          # 2 batches per core
P = BPC * C                # 128 partitions
F = 2048                   # pixels per chunk (1 MiB DMA per chunk)
NCHUNK = N // F            # 32
NCACHE = 16                # chunks kept resident in SBUF for pass 2
MM = 512                   # matmul free-dim tile (one fp32 PSUM bank)
FP32 = mybir.dt.float32
BF16 = mybir.dt.bfloat16

LAST_RESULTS = None
_prog = None


def _build_program():
    nc = bacc.Bacc("TRN2", target_bir_lowering=False, debug=False, num_devices=NCORES)

    x = nc.dram_tensor("x", [P, N], FP32, kind="ExternalInput").ap()
    wt = nc.dram_tensor("wt", [P, P], BF16, kind="ExternalInput").ap()
    w1t = nc.dram_tensor("w1t", [P, 2 * RED], FP32, kind="ExternalInput").ap()
    w2t = nc.dram_tensor("w2t", [2 * RED, P], FP32, kind="ExternalInput").ap()
    browb = nc.dram_tensor("browb", [P, 1], FP32, kind="ExternalInput").ap()
    b1b = nc.dram_tensor("b1b", [2 * RED, 1], FP32, kind="ExternalInput").ap()
    b2b = nc.dram_tensor("b2b", [P, 1], FP32, kind="ExternalInput").ap()
    out = nc.dram_tensor("out", [P, N], FP32, kind="ExternalOutput").ap()

    with tile.TileContext(nc) as tc:
        with (
            tc.tile_pool(name="consts", bufs=1) as consts,
            tc.tile_pool(name="cache", bufs=NCACHE) as cachep,
            tc.tile_pool(name="stream", bufs=5) as streamp,
            tc.tile_pool(name="castp", bufs=3) as castp,
            tc.tile_pool(name="small", bufs=1) as small,
        ):
            # consts go on the scalar (ACT) HWDGE ring so the sync ring can
            # start streaming x immediately
            wt_t = consts.tile([P, P], BF16)
            nc.scalar.dma_start(out=wt_t, in_=wt)
            w1t_t = consts.tile([P, 2 * RED], FP32)
            nc.scalar.dma_start(out=w1t_t, in_=w1t)
            w2t_t = consts.tile([2 * RED, P], FP32)
            nc.scalar.dma_start(out=w2t_t, in_=w2t)
            brow_t = consts.tile([P, 1], FP32)
            nc.scalar.dma_start(out=brow_t, in_=browb)
            b1_t = consts.tile([2 * RED, 1], FP32)
            nc.scalar.dma_start(out=b1_t, in_=b1b)
            b2_t = consts.tile([P, 1], FP32)
            nc.scalar.dma_start(out=b2_t, in_=b2b)

            acc_cols = small.tile([P, NCHUNK], FP32)
            cache_tiles = []

            # ---- pass 1: per chunk, V = Wrow_bd @ x then
            #      acc_cols[:, c] = sum_n x * (V + 1)
            with tc.tile_pool(name="vps", bufs=2, space="PSUM") as vpool:
                for c in range(NCHUNK):
                    if c < NCACHE:
                        xt = cachep.tile([P, F], FP32, tag="xc")
                        cache_tiles.append(xt)
                    else:
                        xt = streamp.tile([P, F], FP32, tag="xs")
                    # alternate the two HWDGE rings: a single ring tops out
                    # ~300 GB/s; both together reach the HBM limit
                    if c % 2 == 0:
                        nc.sync.dma_start(out=xt, in_=x[:, c * F : (c + 1) * F])
                    else:
                        nc.scalar.dma_start(out=xt, in_=x[:, c * F : (c + 1) * F])

                    # bf16 copy of the chunk for the V matmul: single-pass
                    # matmul + fast weight load (fp32 matmul is 2-pass and
                    # was the pass-1 serializer). Only V is quantized; the
                    # sums over x stay f32, and the error is contracted by
                    # the tiny MLP weights + sigmoid, so the output impact
                    # is ~1e-6 relative.
                    # (cast on ACT only: GpSimd CAST measured ~4x slower)
                    xb = castp.tile([P, F], BF16, tag="xb")
                    nc.scalar.copy(xb, xt)

                    vt = vpool.tile([P, F], FP32, tag="v")
                    for s in range(F // MM):
                        nc.tensor.matmul(
                            vt[:, s * MM : (s + 1) * MM],
                            wt_t,
                            xb[:, s * MM : (s + 1) * MM],
                            start=True,
                            stop=True,
                        )
                    # vt = (vt + 1) * xt ; acc_cols[:, c] = sum_free(vt)
                    nc.vector.scalar_tensor_tensor(
                        out=vt,
                        in0=vt,
                        scalar=1.0,
                        in1=xt,
                        op0=mybir.AluOpType.add,
                        op1=mybir.AluOpType.mult,
                        accum_out=acc_cols[:, c : c + 1],
                    )

            # ---- finish: y = acc/n + brow ; z = relu(W1@y + b1) ;
            #      g = sigmoid(W2@z + b2)   (both batches at once)
            acc = small.tile([P, 1], FP32)
            nc.vector.tensor_reduce(
                out=acc, in_=acc_cols, axis=mybir.AxisListType.X, op=mybir.AluOpType.add
            )
            y_t = small.tile([P, 1], FP32)
            nc.scalar.activation(
                out=y_t,
                in_=acc,
                func=mybir.ActivationFunctionType.Identity,
                bias=brow_t,
                scale=1.0 / float(N),
            )
            with tc.tile_pool(name="fps", bufs=1, space="PSUM") as fpool:
                z_ps = fpool.tile([2 * RED, 1], FP32, tag="z")
                nc.tensor.matmul(z_ps, w1t_t, y_t, start=True, stop=True)
                z_t = small.tile([2 * RED, 1], FP32)
                nc.scalar.activation(
                    out=z_t,
                    in_=z_ps,
                    func=mybir.ActivationFunctionType.Relu,
                    bias=b1_t,
                    scale=1.0,
                )
                g_ps = fpool.tile([P, 1], FP32, tag="g")
                nc.tensor.matmul(g_ps, w2t_t, z_t, start=True, stop=True)
                g_t = small.tile([P, 1], FP32)
                nc.scalar.activation(
                    out=g_t,
                    in_=g_ps,
                    func=mybir.ActivationFunctionType.Sigmoid,
                    bias=b2_t,
                    scale=1.0,
                )

            # ---- pass 2: out = x * g (cached chunks from SBUF, rest re-read)
            # Per-partition g is read via a stride-0 broadcast AP: tensor_tensor
            # runs at DVE line rate, while tensor_scalar with an AP scalar hits
            # a ~13x-slower const-pointer-update path. DVE takes 2 of every 3
            # chunks, GpSimd (2-input port-mux floor => ~2x slower) 1 of 3.
            # ACT stays compute-free so its HWDGE ring can stream all stores.
            g_b = g_t.to_broadcast([P, F])
            for c in range(NCHUNK):
                if c < NCACHE:
                    xt = cache_tiles[c]
                else:
                    xt = streamp.tile([P, F], FP32, tag="xs")
                    nc.sync.dma_start(out=xt, in_=x[:, c * F : (c + 1) * F])
                if c % 3 == 0:
                    nc.scalar.mul(xt, xt, g_t)
                elif c % 3 == 1:
                    nc.vector.tensor_mul(xt, xt, g_b)
                else:
                    nc.gpsimd.tensor_mul(xt, xt, g_b)
                nc.scalar.dma_start(out=out[:, c * F : (c + 1) * F], in_=xt)

    nc.compile()
    return nc


def kernel(**inputs) -> np.ndarray:
    global _prog, LAST_RESULTS
    x = np.ascontiguousarray(np.asarray(inputs["x"], dtype=np.float32))
    Wrow = np.asarray(inputs["Wrow"], dtype=np.float32)
    brow = np.asarray(inputs["brow"], dtype=np.float32)
    W1 = np.asarray(inputs["W1"], dtype=np.float32)
    b1 = np.asarray(inputs["b1"], dtype=np.float32)
    W2 = np.asarray(inputs["W2"], dtype=np.float32)
    b2 = np.asarray(inputs["b2"], dtype=np.float32)

    if _prog is None:
        _prog = _build_program()
    nc = _prog

    # Host-side prep: block-diagonal / block layouts so each core's two
    # batches occupy partitions [0:64] and [64:128].
    xr = x.reshape(NCORES, P, N)
    wt_bd = np.zeros((P, P), np.float32)
    wt_bd[:C, :C] = Wrow.T
    wt_bd[C:, C:] = Wrow.T
    wt_bd = wt_bd.astype(ml_dtypes.bfloat16)
    w1t_blk = np.zeros((P, 2 * RED), np.float32)
    w1t_blk[:C, :RED] = W1.T
    w1t_blk[C:, RED:] = W1.T
    w2t_blk = np.zeros((2 * RED, P), np.float32)
    w2t_blk[:RED, :C] = W2.T
    w2t_blk[RED:, C:] = W2.T
    browb = np.tile(brow, BPC).reshape(P, 1).astype(np.float32)
    b1b = np.tile(b1, BPC).reshape(2 * RED, 1).astype(np.float32)
    b2b = np.tile(b2, BPC).reshape(P, 1).astype(np.float32)

    in_maps = [
        dict(
            x=np.ascontiguousarray(xr[i]),
            wt=wt_bd,
            w1t=w1t_blk,
            w2t=w2t_blk,
            browb=browb,
            b1b=b1b,
            b2b=b2b,
        )
        for i in range(NCORES)
    ]
    res = run_bass_kernel_spmd(nc, in_maps, core_ids=list(range(NCORES)))
    LAST_RESULTS = res
    out = np.stack([r["out"] for r in res.results], axis=0)  # [8, 128, N]
    return out.reshape(B, C, H, W)
